# revision 142
# baseline (speedup 1.0000x reference)
"""Trainium2 Bass kernel for causal multi-head attention with RoPE.

Reference computation (B=2, T=2048, D=2048, H=16, dk=128):
    Q = x @ Wq.T ; K = x @ Wk.T ; V = x @ Wv.T          (per-head split)
    Q, K <- RoPE(Q, K)
    attn = softmax(mask(Q K^T / sqrt(dk)))
    out  = (attn @ V) merged-heads @ Wo.T
Sharding (Megatron-style tensor parallel over heads): each of the 8 cores
owns 2 heads (both batches); the host sums the 8 full-shape partials
(the all-reduce after Wo).

Numerics: projections run on the PE in fp8 DoubleRow mode with an hi/lo
split that preserves bf16-level accuracy:
  x ~= x_hi(e4m3) + x_lo(e5m2),  W ~= W_hi(e4m3) + W_lo(e5m2)
  x@W ~= x_hi@W_hi + x_hi@W_lo + x_lo@W_hi      (lo*lo dropped, ~2^-8)
Each term is a DoubleRow matmul (256-deep contraction at 0.5 cycles/row),
so the 3-term product costs 0.75x the bf16 GEMM at bf16-level rel-err.

Schedule (the key perf structure): attention is ACT(exp)-bound per score
tile (~610ns exp vs ~430ns of PE work), so the kernel keeps two deques of
"filler" PE work and pumps them into the in-order PE queue inside the
attention loops -- `fillers` holds mandatory prep (JIT-Q projections,
x(b1) DMA loads, K+V(b1) projection chunks) with priority, and `wo_q`
holds output-projection blocks as the fallback drawn at any stall point:
  P1(b0): stream x(b0), K+V projections (+RoPE on K), DMA order matched
      to the term-major chain consumption order              [PE-dense]
  attn(b0, qt=3..0): per kt tile pump 1 unit (2 in the thin qt<=1
      regions); x(b1) tile nt loads while attn runs on qt=nt, right
      after JIT-Q(b0,qt) released the WAR on the shared x buffer
  attn(b1, qt=1,2,3,0): K/V(b1) late tiles + JIT Qs keep filling; b0's
      last two Wo tiles are held back to feed the b1 tail regions
Everything PE-issued lands in one in-order queue, so fillers are only
emitted where their inputs are already resident.  The softmax denominator
rides in PSUM as 1-column matmuls with exp(scores) stationary; its
reciprocal is reshuffled via 4 tiny DMAs + gpsimd partition-broadcast,
with ~3us of pumped filler covering that latency.  qt==0 regions instead
use a ones-stationary row-broadcast denominator (+213ns/tile on PE) so a
single elementwise reciprocal replaces that whole chain.  PSUM: shared
2-bank projection ring + scores(2) + out-accum(1) + denom(1) + Wo(2) =
8 banks; late Wo gens alternate their psum through the then-idle
projection ring (and the scores ring for the final tile) to deepen the
staging pipeline where only Wo remains.  GPSIMD never touches PSUM (hw
restriction caught by the walrus verifier).  Weights are host-packed to
[p, ko, n] so every weight DMA moves 4KB contiguous per partition line.
"""

import sys

sys.path.insert(0, "/opt/trn_rl_repo")

from collections import deque

import numpy as np
import ml_dtypes

import concourse.bass as bass  # noqa: F401  (registers engine classes)
import concourse.mybir as mybir
import concourse.tile as tile
from concourse import bacc
from concourse.bass_utils import run_bass_kernel_spmd

BF16 = ml_dtypes.bfloat16
E4 = ml_dtypes.float8_e4m3
E5 = ml_dtypes.float8_e5m2

B, T, D, H = 2, 2048, 2048, 16
DK = D // H          # 128
THETA = 10000.0
NCORES = 8
HL = H // NCORES     # 2 local heads per core
DLOC = HL * DK       # 256 local output dims per projection
TOK = B * T          # 4096
P = 128
KD = D // P          # 16 contraction tiles
QT_PER_B = T // 512  # 4 query tiles per batch
NTB = T // 512       # 4 token tiles of 512 per batch
TB = T // P          # 16 token blocks of 128 per batch
SCALE = 1.0 / float(np.sqrt(DK))

_dt = mybir.dt
DR = mybir.MatmulPerfMode.DoubleRow


def _build_kernel():
    nc = bacc.Bacc("TRN2", target_bir_lowering=False, debug=False,
                   num_devices=NCORES)

    XH = nc.dram_tensor("XH", [D, TOK], _dt.float8e4, kind="ExternalInput")
    XL = nc.dram_tensor("XL", [D, TOK], _dt.float8e5, kind="ExternalInput")
    # weights host-packed [p, ko, n] so DMA lines are 4KB contiguous
    WQH = nc.dram_tensor("WQH", [P, KD, DLOC], _dt.float8e4,
                         kind="ExternalInput")
    WQL = nc.dram_tensor("WQL", [P, KD, DLOC], _dt.float8e5,
                         kind="ExternalInput")
    WKH = nc.dram_tensor("WKH", [P, KD, DLOC], _dt.float8e4,
                         kind="ExternalInput")
    WKL = nc.dram_tensor("WKL", [P, KD, DLOC], _dt.float8e5,
                         kind="ExternalInput")
    WVH = nc.dram_tensor("WVH", [P, KD, DLOC], _dt.float8e4,
                         kind="ExternalInput")
    WVL = nc.dram_tensor("WVL", [P, KD, DLOC], _dt.float8e5,
                         kind="ExternalInput")
    WOH = nc.dram_tensor("WOH", [P, HL, D], _dt.float8e4,
                         kind="ExternalInput")
    WOL = nc.dram_tensor("WOL", [P, HL, D], _dt.float8e5,
                         kind="ExternalInput")
    COS = nc.dram_tensor("COS", [P, T], _dt.bfloat16, kind="ExternalInput")
    SIN = nc.dram_tensor("SIN", [P, T], _dt.bfloat16, kind="ExternalInput")
    ONES = nc.dram_tensor("ONES", [P, P], _dt.bfloat16, kind="ExternalInput")
    MD = nc.dram_tensor("MD", [P, 4, 512], _dt.bfloat16, kind="ExternalInput")
    # bf16 partials: halves the output DMA; host accumulates in fp32
    yT = nc.dram_tensor("yT", [D, TOK], _dt.bfloat16, kind="ExternalOutput")

    xh_r = XH.ap().rearrange("(ko p) m -> p ko m", p=P)    # [128, 16, 4096]
    xl_r = XL.ap().rearrange("(ko p) m -> p ko m", p=P)

    with tile.TileContext(nc) as tc:
        with (
            tc.tile_pool(name="const", bufs=1) as cp,
            tc.tile_pool(name="data", bufs=1) as dp,
            tc.tile_pool(name="work", bufs=3) as wp,
            tc.tile_pool(name="psproj", bufs=2, space="PSUM") as pj,
            tc.tile_pool(name="psatt", bufs=1, space="PSUM") as pa,
            tc.tile_pool(name="psy", bufs=2, space="PSUM") as py,
        ):
            wqh_sb = cp.tile([P, KD, DLOC], _dt.float8e4, tag="wqh")
            wql_sb = cp.tile([P, KD, DLOC], _dt.float8e5, tag="wql")
            wkh_sb = cp.tile([P, KD, DLOC], _dt.float8e4, tag="wkh")
            wkl_sb = cp.tile([P, KD, DLOC], _dt.float8e5, tag="wkl")
            wvh_sb = cp.tile([P, KD, DLOC], _dt.float8e4, tag="wvh")
            wvl_sb = cp.tile([P, KD, DLOC], _dt.float8e5, tag="wvl")
            woh_sb = cp.tile([P, HL, D], _dt.float8e4, tag="woh")
            wol_sb = cp.tile([P, HL, D], _dt.float8e5, tag="wol")
            cos_sb = cp.tile([P, T], _dt.bfloat16, tag="cos")
            sin_sb = cp.tile([P, T], _dt.bfloat16, tag="sin")
            ones_sb = cp.tile([P, P], _dt.bfloat16, tag="ones")
            md_sb = cp.tile([P, 4, 512], _dt.bfloat16, tag="md")

            # single-batch x residency (reused b0 -> b1; WAR release order
            # is arranged by the JIT-Q consumption / descending b1 loads)
            xh_sb = dp.tile([P, KD, T], _dt.float8e4, tag="xh")
            xl_sb = dp.tile([P, KD, T], _dt.float8e5, tag="xl")
            # per-batch double-buffered activations (partition = head dim;
            # RoPE applied in place)
            qt_sb = [dp.tile([P, HL, T], _dt.bfloat16, tag=f"qt{i}",
                             name=f"qt{i}") for i in range(2)]
            kt_sb = [dp.tile([P, HL, T], _dt.bfloat16, tag=f"kt{i}",
                             name=f"kt{i}") for i in range(2)]
            v_sb = [dp.tile([P, TB, DLOC], _dt.bfloat16, tag=f"v{i}",
                            name=f"v{i}") for i in range(2)]

            # ---------------- emission helpers (generators) ----------------
            fillers = deque()   # mandatory prep work (q/x/kv), FIFO
            wo_q = deque()      # Wo blocks: fallback filler for any stall

            def pump(units):
                while units > 0:
                    src = fillers if fillers else wo_q
                    if not src:
                        return
                    try:
                        if next(src[0]) != "dma":
                            units -= 1
                    except StopIteration:
                        src.popleft()

            def pump_wo(units):
                while units > 0 and wo_q:
                    try:
                        if next(wo_q[0]) != "dma":
                            units -= 1
                    except StopIteration:
                        wo_q.popleft()

            def force(gen):
                for _ in gen:
                    pass

            def forcep(gen):
                """Force-drain a gate generator while pumping deque filler
                between its units, so ring stalls in the dense drain are
                covered by other ready work."""
                try:
                    fillers.remove(gen)
                except ValueError:
                    pass
                for v in gen:
                    if v != "dma":
                        pump(1)

            def drain_all():
                while fillers:
                    force(fillers.popleft())

            def gen_xload(b, nt):
                ts0 = b * T + nt * 512
                d0 = nt * 512
                nc.sync.dma_start(xh_sb[:, :, d0:d0 + 512],
                                  xh_r[:, :, ts0:ts0 + 512])
                yield "dma"
                nc.sync.dma_start(xl_sb[:, :, d0:d0 + 512],
                                  xl_r[:, :, ts0:ts0 + 512])
                yield "dma"

            def rope_tail(dst_t, m, c0, dq=None):
                """In-place RoPE tail: the pairwise partition rotation is
                done by two partition-strided SBUF->SBUF DMAs (the rotation
                signs are folded into the host-prepped sin table), freeing
                the PE of the rot matmuls and making t2 an all-SBUF bf16
                4x-mode DVE multiply."""
                sl = dst_t[:, m, c0:c0 + 512]
                swp = wp.tile([P, 512], _dt.bfloat16, tag="swp", bufs=2,
                              name="swp")
                dq = dq or nc.sync
                dq.dma_start(swp[0:P:2, :], dst_t[1:P:2, m, c0:c0 + 512])
                dq.dma_start(swp[1:P:2, :], dst_t[0:P:2, m, c0:c0 + 512])
                t1 = wp.tile([P, 512], _dt.bfloat16, tag="t1", bufs=2, name="t1")
                nc.vector.tensor_mul(t1[:], sl, cos_sb[:, c0:c0 + 512])
                t2 = wp.tile([P, 512], _dt.bfloat16, tag="t2", bufs=2, name="t2")
                nc.vector.tensor_mul(t2[:], swp[:], sin_sb[:, c0:c0 + 512])
                nc.gpsimd.tensor_add(sl, t1[:], t2[:])

            def gen_proj(wh_sb, wl_sb, dst, nt, par, term_major=False):
                """K or Q projection of token tile nt (512 tokens) for both
                local heads, RoPE'd in place into dst[par].  term_major
                orders the 3-term chains to match the P1 DMA arrival order
                (xh/wh first, wl, then xl)."""
                d0 = nt * 512
                ps = {}
                if term_major:
                    for m in range(HL):
                        ps[m] = pj.tile([P, 512], _dt.float32, tag="pj",
                                        name=f"psp{m}")
                    for ti, (xs, ws) in enumerate(((xh_sb, wh_sb),
                                                   (xh_sb, wl_sb),
                                                   (xl_sb, wh_sb))):
                        for m in range(HL):
                            ms = slice(m * P, (m + 1) * P)
                            for j in range(KD // 2):
                                js = slice(2 * j, 2 * j + 2)
                                nc.tensor.matmul(
                                    ps[m][:], ws[:, js, ms],
                                    xs[:, js, d0:d0 + 512],
                                    start=(ti == 0 and j == 0),
                                    stop=(ti == 2 and j == KD // 2 - 1),
                                    perf_mode=DR)
                                if j % 3 == 2:
                                    yield
                            if ti == 2:
                                sl = dst[par][:, m, d0:d0 + 512]
                                nc.scalar.copy(sl, ps[m][:])
                                yield
                    for m in range(HL):
                        rope_tail(dst[par], m, d0)
                        yield
                else:
                    for m in range(HL):
                        psm = pj.tile([P, 512], _dt.float32, tag="pj",
                                      name="psp")
                        ms = slice(m * P, (m + 1) * P)
                        for xs, ws, first, last in (
                                (xh_sb, wh_sb, True, False),
                                (xl_sb, wh_sb, False, False),
                                (xh_sb, wl_sb, False, True)):
                            for j in range(KD // 2):
                                js = slice(2 * j, 2 * j + 2)
                                nc.tensor.matmul(
                                    psm[:], ws[:, js, ms],
                                    xs[:, js, d0:d0 + 512],
                                    start=(first and j == 0),
                                    stop=(last and j == KD // 2 - 1),
                                    perf_mode=DR)
                                if j % 3 == 2:
                                    yield
                        sl = dst[par][:, m, d0:d0 + 512]
                        nc.scalar.copy(sl, psm[:])
                        yield
                        rope_tail(dst[par], m, d0)
                        yield

            def gen_v(nt, par, vp=None):
                """V projection of token tile nt in natural [tok, dloc]
                layout (tokens stationary, weights moving).  vp selects the
                psum pool: py during P1 (decoupled from the K ring, Wo gens
                not yet active), pj for the filler instances."""
                for tb in range(4 * nt, 4 * nt + 4):
                    tbs = slice(tb * P, tb * P + P)
                    if vp is py:
                        psv = py.tile([P, DLOC], _dt.float32, tag="y",
                                      name="psv")
                    else:
                        psv = pj.tile([P, DLOC], _dt.float32, tag="pj",
                                      name="psv")
                    for xs, ws, first, last in (
                            (xh_sb, wvh_sb, True, False),
                            (xh_sb, wvl_sb, False, False),
                            (xl_sb, wvh_sb, False, True)):
                        for j in range(KD // 2):
                            js = slice(2 * j, 2 * j + 2)
                            nc.tensor.matmul(
                                psv[:], xs[:, js, tbs],
                                ws[:, js, :],
                                start=(first and j == 0),
                                stop=(last and j == KD // 2 - 1),
                                perf_mode=DR)
                            if j % 3 == 2:
                                yield
                    nc.scalar.copy(v_sb[par][:, tb, :], psv[:])
                    yield

            def gen_wo(b, qt, ot8h, ot8l, late=False, alt_ring=False):
                """Output projection of one query tile: 16 nb blocks of
                3-term fp8 DR matmuls, staged to bf16 and DMA'd per 4.
                late: stage copies on ACT/Pool (DVE is congested where these
                gens drain).  alt_ring: also rotate psum through the scores
                pool (only safe once all attention is done)."""
                q0 = b * T + qt * 512
                for nbg in range(4):
                    ysb = wp.tile([P, 4, 512], _dt.bfloat16,
                                  tag="ysb", bufs=4, name="ysb")
                    for i in range(4):
                        nb = nbg * 4 + i
                        nbs = slice(nb * P, (nb + 1) * P)
                        if alt_ring and i % 2 == 1:
                            yp = pa.tile([P, 512], _dt.float32, tag="s",
                                         bufs=2, name="yps")
                        elif late and i % 2 == 1:
                            yp = pj.tile([P, 512], _dt.float32, tag="pj",
                                         name="ypj")
                        else:
                            yp = py.tile([P, 512], _dt.float32, tag="y",
                                         name="yp")
                        nc.tensor.matmul(yp[:], woh_sb[:, :, nbs],
                                         ot8h[:], start=True,
                                         stop=False, perf_mode=DR)
                        nc.tensor.matmul(yp[:], woh_sb[:, :, nbs],
                                         ot8l[:], start=False,
                                         stop=False, perf_mode=DR)
                        nc.tensor.matmul(yp[:], wol_sb[:, :, nbs],
                                         ot8h[:], start=False,
                                         stop=True, perf_mode=DR)
                        # PSUM is only readable by ACT/DVE (never gpsimd)
                        if (i % 2 == 0) if late else (i == 2):
                            nc.scalar.copy(ysb[:, i, :], yp[:])
                        else:
                            nc.vector.tensor_copy(ysb[:, i, :], yp[:])
                        yield
                    # half-group DMAs: shorter transfers reduce SP-queue
                    # head-of-line blocking of the latency-critical rcTf
                    # chain and drain the tail backlog sooner
                    for h in range(2):
                        nc.sync.dma_start(
                            yT[nbg * 512 + h * 256:nbg * 512 + h * 256 + 256,
                               q0:q0 + 512]
                            .rearrange("(i p) q -> p i q", p=P),
                            ysb[:, 2 * h:2 * h + 2, :])
                    yield

            # ------------------------- attention --------------------------
            def attn(b, qt, par):
                nk = (qt + 1) * 4
                ot8h = wp.tile([P, HL, 512], _dt.float8e4, tag="ot8h",
                               bufs=7, name=f"ot8h_{b}_{qt}")
                ot8l = wp.tile([P, HL, 512], _dt.float8e5, tag="ot8l",
                               bufs=7, name=f"ot8l_{b}_{qt}")
                q0 = qt * 512
                # qt==0 regions are filler-thin: spend ~213ns/tile of PE on
                # a ones-stationary row-broadcast denominator instead of the
                # free column one, killing the reciprocal-transpose-DMA
                # latency chain at each hl boundary
                dn_rows = (qt == 0)
                for hl in range(HL):
                    op = pa.tile([P, 512], _dt.float32, tag="o", bufs=1)
                    dn = pa.tile([P, 512 if dn_rows else 4], _dt.float32,
                                 tag="dn", bufs=1)

                    # software-pipelined: emit tile kt's QK/exp two steps
                    # ahead of its PV/dn so the PE never waits on the ACT
                    # exp latency
                    def emit_qk(kt):
                        j = kt - 4 * qt
                        qoff = max(j, 0) * P
                        nq = 512 - qoff
                        sp_ = pa.tile([P, 512], _dt.float32, tag="s",
                                      bufs=2, name=f"s_{b}_{hl}_{kt}")
                        nc.tensor.matmul(
                            sp_[:, :nq],
                            kt_sb[par][:, hl, kt * P:(kt + 1) * P],
                            qt_sb[par][:, hl, q0 + qoff:q0 + 512],
                            start=True, stop=True)
                        pT = wp.tile([P, 512], _dt.bfloat16, tag="pT",
                                     bufs=5, name=f"p_{b}_{hl}_{kt}")
                        nc.scalar.activation(
                            pT[:, :nq], sp_[:, :nq],
                            mybir.ActivationFunctionType.Exp,
                            scale=SCALE)
                        if j >= 0:  # 0/1 mask inside the diagonal
                            nc.vector.tensor_mul(pT[:, :nq], pT[:, :nq],
                                                 md_sb[:, j, qoff:])
                        return pT, qoff, nq

                    def emit_pv(kt, pT, qoff, nq):
                        j = kt - 4 * qt
                        st = (kt == 0)
                        nc.tensor.matmul(
                            op[:, qoff:],
                            v_sb[par][:, kt, hl * P:(hl + 1) * P],
                            pT[:, :nq], start=st, stop=(kt == nk - 1))
                        if dn_rows:
                            # all 128 output rows = the key-sum of pT
                            nc.tensor.matmul(
                                dn[:, qoff:], ones_sb[:],
                                pT[:, :nq], start=st, stop=(j == 3),
                                skip_group_check=True)
                            return
                        # denominator: pT stationary, ones moving.  A
                        # start=True matmul zeroes the WHOLE psum bank, so
                        # only the very first chunk write may carry it; the
                        # other kt==0 chunks land on pending-zero bytes and
                        # still overwrite.  One stop on the last instr.
                        for c in range(qoff // P, 4):
                            nc.tensor.matmul(
                                dn[:, c:c + 1],
                                pT[:, c * P - qoff:c * P - qoff + P],
                                ones_sb[:, 0:1],
                                start=(st and c == 0),
                                stop=(j == 3 and c == 3),
                                skip_group_check=True)

                    depth = 2
                    pump(6)
                    fifo = [emit_qk(kt) for kt in range(depth)]
                    for kt in range(depth, nk):
                        fifo.append(emit_qk(kt))
                        emit_pv(kt - depth, *fifo.pop(0))
                        pump(1 if qt >= 2 else 2)
                    for d in range(depth):
                        emit_pv(nk - depth + d, *fifo.pop(0))
                        pump(1 if qt >= 2 else 2)

                    if dn_rows:
                        # dn already holds the key-sum broadcast across all
                        # partitions: one elementwise reciprocal finishes it
                        rbF = wp.tile([P, 512], _dt.float32, tag="rbF",
                                      bufs=1, name="rbF")
                        nc.vector.reciprocal(rbF[:], dn[:])
                        rbS = rbF
                        pump(6)
                    else:
                        # denominator reciprocal, then partition-major ->
                        # free-major reshuffle and broadcast entirely on DMA
                        # + the idle gpsimd queue; pumped filler keeps the
                        # PE fed while this chain is in flight
                        rcf = wp.tile([P, 4], _dt.float32, tag="rcf",
                                      name="rcf")
                        nc.vector.reciprocal(rcf[:], dn[:])
                        rcb = wp.tile([P, 4], _dt.bfloat16, tag="rcb",
                                      name="rcb")
                        nc.vector.tensor_copy(rcb[:], rcf[:])
                        rcTf = wp.tile([1, 512], _dt.bfloat16, tag="rcTf",
                                       bufs=2, name="rcTf")
                        for c in range(4):
                            nc.sync.dma_start(rcTf[0:1, c * P:(c + 1) * P],
                                              rcb[:, c:c + 1])
                        rbS = wp.tile([P, 512], _dt.bfloat16, tag="rbS",
                                      bufs=3, name="rbS")
                        nc.gpsimd.partition_broadcast(rbS[:], rcTf[0:1, :])
                        pump(10)
                    # normalized head output in fp8 hi/lo for the DoubleRow
                    # output projection
                    t3 = wp.tile([P, 512], _dt.bfloat16, tag="t3", bufs=3,
                                 name="t3")
                    nc.vector.tensor_mul(t3[:], op[:], rbS[:])
                    nc.vector.tensor_copy(ot8h[:, hl, :], t3[:])
                    nc.vector.tensor_sub(ot8l[:, hl, :], t3[:],
                                         ot8h[:, hl, :])
                return ot8h, ot8l

            # ======================= emission order =======================
            # P1(b0): x(b0) stream + K,V projections (+RoPE K), PE-dense.
            # DMA order matched to the term-major K-chain consumption order
            # (xh&wh chunks, then wl & xl chunks, then wv); weights for
            # later phases deferred behind all x tiles.
            for kc in range(0, KD, 4):
                nc.sync.dma_start(wkh_sb[:, kc:kc + 4, :],
                                  WKH[:, kc:kc + 4, :])
                nc.sync.dma_start(xh_sb[:, kc:kc + 4, 0:512],
                                  xh_r[:, kc:kc + 4, 0:512])
            nc.sync.dma_start(wkl_sb[:], WKL[:])
            nc.sync.dma_start(wvh_sb[:], WVH[:])
            for kc in range(0, KD, 4):
                nc.sync.dma_start(xl_sb[:, kc:kc + 4, 0:512],
                                  xl_r[:, kc:kc + 4, 0:512])
            nc.sync.dma_start(wvl_sb[:], WVL[:])
            force(gen_xload(0, 1))
            force(gen_xload(0, 2))
            nc.sync.dma_start(cos_sb[:], COS[:])
            nc.sync.dma_start(sin_sb[:], SIN[:])
            force(gen_xload(0, 3))
            nc.sync.dma_start(wqh_sb[:], WQH[:])
            nc.sync.dma_start(wql_sb[:], WQL[:])
            nc.sync.dma_start(ones_sb[:], ONES[:])
            nc.sync.dma_start(md_sb[:], MD[:])
            nc.sync.dma_start(woh_sb[:], WOH[:])
            nc.sync.dma_start(wol_sb[:], WOL[:])

            # compute order K0,V0,K1,V1,... tracks the x stream (V uses no
            # new data, so each V block is catch-up slack for the DMA pipe).
            # JIT-Q(b0, qt3) goes before the last V block so its RoPE chain
            # latency hides under V3.
            for nt in range(NTB):
                force(gen_proj(wkh_sb, wkl_sb, kt_sb, nt, 0,
                               term_major=True))
                if nt == NTB - 1:
                    force(gen_proj(wqh_sb, wql_sb, qt_sb, 3, 0))
                force(gen_v(nt, 0))

            # schedule: (b, qt, gate-gens, filler-gens).  b0 runs qt
            # descending; b1 runs [1,2,3,0] so the K/V(b1) late tiles and
            # the JIT Qs fill b1's attention regions too.  x(b1) slice nt
            # is WAR-free right after JIT-Q(b0, qt=nt) has been emitted.
            def kv1(nt):
                return [gen_xload(1, nt),
                        gen_proj(wkh_sb, wkl_sb, kt_sb, nt, 1),
                        gen_v(nt, 1)]

            q_ = {(b, qt): gen_proj(wqh_sb, wql_sb, qt_sb, qt, b)
                  for b in range(B) for qt in range(QT_PER_B)
                  if (b, qt) != (0, 3)}
            x1 = {nt: gen_xload(1, nt) for nt in range(NTB)}
            k1 = {nt: gen_proj(wkh_sb, wkl_sb, kt_sb, nt, 1)
                  for nt in range(NTB)}
            v1 = {nt: gen_v(nt, 1) for nt in range(NTB)}

            sched = [
                (0, 3, [],
                 [q_[(0, 2)], x1[3]]),
                (0, 2, [q_[(0, 2)]],
                 [q_[(0, 1)], x1[2]]),
                (0, 1, [q_[(0, 1)]],
                 [q_[(0, 0)], x1[1]]),
                (0, 0, [q_[(0, 0)]],
                 [x1[0], k1[1], v1[1], q_[(1, 1)], k1[0], v1[0]]),
                (1, 1, [x1[0], k1[1], v1[1], k1[0], v1[0], q_[(1, 1)]],
                 [k1[2], v1[2], q_[(1, 2)]]),
                (1, 2, [k1[2], v1[2], q_[(1, 2)]],
                 [k1[3], v1[3], q_[(1, 3)]]),
                (1, 3, [k1[3], v1[3], q_[(1, 3)]],
                 [q_[(1, 0)]]),
                (1, 0, [q_[(1, 0)]],
                 []),
            ]
            # b0's last two Wo tiles are held back to feed the b1 tail
            # regions, which otherwise run out of filler (needs the deep
            # ot8 ring above)
            hold = {(0, 1): (1, 1), (0, 0): (1, 1)}
            wo_held = {}
            for b, qt, gates, fills in sched:
                for g in gates:
                    force(g)
                for g in fills:
                    fillers.append(g)
                for k, tgt in list(hold.items()):
                    if tgt == (b, qt) and k in wo_held:
                        wo_q.append(wo_held.pop(k))
                ot8h, ot8l = attn(b, qt, b)
                late = b == 1 or (b, qt) in hold
                g = gen_wo(b, qt, ot8h, ot8l, late=late,
                           alt_ring=(b == 1 and qt == 0))
                if (b, qt) in hold:
                    wo_held[(b, qt)] = g
                else:
                    wo_q.append(g)
            drain_all()
            while wo_q:
                force(wo_q.popleft())

    nc.compile()
    return nc


_NC_CACHE = None


def _get_nc():
    global _NC_CACHE
    if _NC_CACHE is None:
        _NC_CACHE = _build_kernel()
    return _NC_CACHE


def _rope_tables():
    inv_freq = 1.0 / THETA ** (np.arange(0, DK, 2, dtype=np.float32) / DK)
    t = np.arange(T, dtype=np.float32)
    freqs = np.outer(t, inv_freq)                 # (T, dk/2)
    freqs = np.repeat(freqs, 2, axis=-1)          # (T, dk)
    return np.cos(freqs), np.sin(freqs)


def _hi_lo(a):
    """fp32 -> (e4m3 hi, e5m2 lo) split, as contiguous arrays."""
    hi = np.ascontiguousarray(a).astype(E4)
    lo = (a - hi.astype(np.float32)).astype(E5)
    return hi, lo


def _pack_w(w):
    """[D, DLOC] (rows = ko*128+p) -> [P, KD, DLOC] contiguous."""
    return np.ascontiguousarray(
        w.reshape(KD, P, DLOC).transpose(1, 0, 2))


def _host_inputs(x, Wq, Wk, Wv, Wo):
    """Build the per-core input maps (all host-side prep is free)."""
    xT = np.ascontiguousarray(x.reshape(TOK, D).T)   # [D, B*T] fp32
    xh, xl = _hi_lo(xT)
    cos, sin = _rope_tables()                        # (T, dk)
    cosT = np.ascontiguousarray(cos.T).astype(BF16)  # [128, T]
    sinT = np.ascontiguousarray(sin.T).astype(BF16)

    # rotation signs folded into the sin table: the device computes
    # t2[d] = x[d^1] * sin'[d] with sin'[2i] = -sin[2i]
    sinT[0::2, :] = -sinT[0::2, :]
    ones = np.ones((P, P), dtype=BF16)

    # diagonal-block masks, scores layout [key, query]; offset j*128
    md = np.zeros((4, P, 512), dtype=np.float32)
    kk = np.arange(P)[:, None]
    qq = np.arange(512)[None, :]
    for j in range(4):
        md[j] = (qq >= kk + j * P).astype(np.float32)
    md = np.ascontiguousarray(md.transpose(1, 0, 2)).astype(BF16)

    in_maps = []
    for c in range(NCORES):
        rows = slice(c * DLOC, (c + 1) * DLOC)
        wqh, wql = _hi_lo(Wq[rows, :].T)
        wkh, wkl = _hi_lo(Wk[rows, :].T)
        wvh, wvl = _hi_lo(Wv[rows, :].T)
        woh, wol = _hi_lo(Wo[:, rows].T)   # [DLOC, D], rows = ho*128+p
        woh = np.ascontiguousarray(
            woh.reshape(HL, P, D).transpose(1, 0, 2))
        wol = np.ascontiguousarray(
            wol.reshape(HL, P, D).transpose(1, 0, 2))
        in_maps.append({
            "XH": xh, "XL": xl,
            "WQH": _pack_w(wqh), "WQL": _pack_w(wql),
            "WKH": _pack_w(wkh), "WKL": _pack_w(wkl),
            "WVH": _pack_w(wvh), "WVL": _pack_w(wvl),
            "WOH": woh, "WOL": wol,
            "COS": cosT, "SIN": sinT, "ONES": ones,
            "MD": md,
        })
    return in_maps


def _run(in_maps, **kwargs):
    nc = _get_nc()
    return run_bass_kernel_spmd(nc, in_maps, core_ids=list(range(NCORES)),
                                **kwargs)


def kernel(x, Wq, Wk, Wv, Wo, mask, _bench_results=None, **_kw):
    x = np.asarray(x, dtype=np.float32)
    Wq = np.asarray(Wq, dtype=np.float32)
    Wk = np.asarray(Wk, dtype=np.float32)
    Wv = np.asarray(Wv, dtype=np.float32)
    Wo = np.asarray(Wo, dtype=np.float32)
    mask = np.asarray(mask)
    causal = np.array_equal(mask.reshape(T, T),
                            np.tril(np.ones((T, T), dtype=bool)))
    if not causal:
        raise NotImplementedError("kernel specialized for the causal mask")

    res = _run(_host_inputs(x, Wq, Wk, Wv, Wo))
    if _bench_results is not None:
        _bench_results.append(res)

    acc = np.zeros((D, TOK), dtype=np.float32)
    for r in res.results:
        acc += r["yT"].astype(np.float32)
    # yT[n, b*T + t] -> out[b, t, n]
    return np.ascontiguousarray(acc.reshape(D, B, T).transpose(1, 2, 0))
